# revision 5
# baseline (speedup 1.0000x reference)
"""AngularPenaltySMLoss (CosFace) on 8 TRN2 NeuronCores.

Strategy: tensor-parallel over the class dimension C=100000. Each core owns
12500 classes (zero-padded to 12800 = 25 c-tiles of 512). Per core:
  - logits tile [128 n x 512 c] = bf16 matmul of xT (stationary) against
    W-shard^T (moving), K=512 contracted in 4 accumulating PSUM steps
  - ScalarE Exp activation straight from PSUM with per-partition scale
    a[n] = S/||x_n|| (folds the L2 normalization and the S factor into the
    activation) and the fused row-sum accumulator -> per-sample partial
    exp-sums. Each zero pad class contributes exactly exp(0)=1; the total
    padding count is subtracted as a constant in the epilogue.
  - target logits from host-gathered W[labels] rows (data movement only):
    one fused DVE multiply+row-reduce per n-tile.
  - AllReduce (16 KB) of the per-sample partial sums, then the CosFace
    log/denominator epilogue on-device; every core emits the same scalar.
"""

import numpy as np

from concourse import bacc, mybir, tile
from concourse.bass_utils import run_bass_kernel_spmd

N, D, C = 4096, 512, 100000
N_CORES = 8
C_SHARD = C // N_CORES          # 12500
CT = 512                        # class-tile width (one PSUM bank of f32)
NJ = 25                         # c-tiles per core -> C_PAD = 12800
S = 30.0
SM = 10.5                       # S * margin(0.35)

f32 = mybir.dt.float32
bf16 = mybir.dt.bfloat16
AF = mybir.ActivationFunctionType
ALU = mybir.AluOpType
AX = mybir.AxisListType


def build(n=N, d=D, c_shard=C_SHARD, ct=CT, nj=NJ, n_cores=N_CORES):
    ni = n // 128
    nk = d // 128
    c_pad = nj * ct
    pad_corr = float(n_cores * (c_pad - c_shard))

    nc = bacc.Bacc("TRN2", target_bir_lowering=False, debug=False,
                   num_devices=n_cores)
    x_nat = nc.dram_tensor("x_nat", [n, d], f32, kind="ExternalInput").ap()
    xt = nc.dram_tensor("xt", [d, n], f32, kind="ExternalInput").ap()
    wl = nc.dram_tensor("wl", [n, d], f32, kind="ExternalInput").ap()
    wt = nc.dram_tensor("wt", [d, c_pad], f32, kind="ExternalInput").ap()
    out = nc.dram_tensor("out", [1, 1], f32, kind="ExternalOutput").ap()

    with tile.TileContext(nc) as tc:
        with (
            tc.tile_pool(name="persist", bufs=1) as pp,
            tc.tile_pool(name="stage", bufs=3) as sp,
            tc.tile_pool(name="wstage", bufs=6) as wsp,
            tc.tile_pool(name="wbuf", bufs=10) as wbp,
            tc.tile_pool(name="scr", bufs=2) as scp,
            tc.tile_pool(name="psum", bufs=6, space="PSUM") as psp,
            tc.tile_pool(name="psum1", bufs=1, space="PSUM") as psp1,
            tc.tile_pool(name="dram", bufs=1, space="DRAM") as dp,
        ):
            xtb = [pp.tile([128, n], bf16, tag=f"xtb{k}", name=f"xtb{k}")
                   for k in range(nk)]
            parts = pp.tile([128, ni * nj], f32, tag="parts", name="parts")
            ss = pp.tile([128, ni], f32, tag="ss", name="ss")
            tgt = pp.tile([128, ni], f32, tag="tgt", name="tgt")
            u = pp.tile([128, ni], f32, tag="u", name="u")
            a_all = pp.tile([128, ni], f32, tag="a_all", name="a_all")
            tot = pp.tile([128, ni], f32, tag="tot", name="tot")
            loc = pp.tile([128, ni], f32, tag="loc", name="loc")
            ones = pp.tile([128, 1], f32, tag="ones", name="ones")

            # xT resident in SBUF as bf16 (stationary matmul operand)
            for k in range(nk):
                xts = sp.tile([128, n], f32, tag="xts", name="xts")
                nc.sync.dma_start(xts[:], xt[k * 128:(k + 1) * 128, :])
                nc.vector.tensor_copy(xtb[k][:], xts[:])

            # per-sample ||x||^2 (ACT square w/ accumulate) and target raw
            # logit dot(x_n, W[label_n]) (fused DVE mul+reduce)
            for i in range(ni):
                xa = sp.tile([128, d], f32, tag="xa", name="xa")
                nc.sync.dma_start(xa[:], x_nat[i * 128:(i + 1) * 128, :])
                sq = scp.tile([128, d], f32, tag="sq", name="sq")
                nc.scalar.activation(sq[:], xa[:], AF.Square,
                                     accum_out=ss[:, i:i + 1])
                wla = sp.tile([128, d], f32, tag="wla", name="wla")
                nc.sync.dma_start(wla[:], wl[i * 128:(i + 1) * 128, :])
                pr = scp.tile([128, d], f32, tag="pr", name="pr")
                # (tensor_tensor_reduce is a custom DVE ISA op that faults
                # the exec unit on this runtime -- use mul + reduce instead)
                nc.vector.tensor_mul(pr[:], xa[:], wla[:])
                nc.vector.reduce_sum(tgt[:, i:i + 1], pr[:], axis=AX.X)

            # a[n] = S / ||x_n|| = 1 / sqrt(ss / S^2)
            nc.scalar.activation(u[:], ss[:], AF.Sqrt, scale=1.0 / (S * S))
            nc.vector.reciprocal(a_all[:], u[:])
            nc.vector.memset(ones[:], 1.0)

            # main loop: nj c-tiles x ni n-tiles
            for j in range(nj):
                wbt = []
                for k in range(nk):
                    wf = wsp.tile([128, ct], f32, tag="wf", name="wf")
                    nc.sync.dma_start(
                        wf[:], wt[k * 128:(k + 1) * 128, j * ct:(j + 1) * ct])
                    wb = wbp.tile([128, ct], bf16, tag="wb", name="wb")
                    nc.vector.tensor_copy(wb[:], wf[:])
                    wbt.append(wb)
                for i in range(ni):
                    ps = psp.tile([128, ct], f32, tag="ps", name="ps")
                    for k in range(nk):
                        nc.tensor.matmul(ps[:],
                                         xtb[k][:, i * 128:(i + 1) * 128],
                                         wbt[k][:],
                                         start=(k == 0), stop=(k == nk - 1))
                    es = scp.tile([128, ct], bf16, tag="es", name="es")
                    col = i * nj + j
                    nc.scalar.activation(es[:], ps[:], AF.Exp,
                                         scale=a_all[:, i:i + 1],
                                         accum_out=parts[:, col:col + 1])

            # per-sample local sum over this core's nj c-tiles
            for i in range(ni):
                nc.vector.reduce_sum(loc[:, i:i + 1],
                                     parts[:, i * nj:(i + 1) * nj], axis=AX.X)

            cc_in = dp.tile([128, ni], f32, name="cc_in")
            cc_out = dp.tile([128, ni], f32, addr_space="Shared", name="cc_out")
            nc.sync.dma_start(cc_in[:], loc[:])
            nc.gpsimd.collective_compute(
                "AllReduce", ALU.add,
                replica_groups=[list(range(n_cores))],
                ins=[cc_in[:]], outs=[cc_out[:]])
            nc.sync.dma_start(tot[:], cc_out[:])

            # epilogue: loss = mean(log(den) - S*tgt) + S*margin
            t1 = pp.tile([128, ni], f32, tag="t1", name="t1")
            e1 = pp.tile([128, ni], f32, tag="e1", name="e1")
            e2 = pp.tile([128, ni], f32, tag="e2", name="e2")
            den = pp.tile([128, ni], f32, tag="den", name="den")
            lg = pp.tile([128, ni], f32, tag="lg", name="lg")
            v = pp.tile([128, ni], f32, tag="v", name="v")
            rowv = pp.tile([128, 1], f32, tag="rowv", name="rowv")
            res = pp.tile([1, 1], f32, tag="res", name="res")

            nc.vector.tensor_mul(t1[:], a_all[:], tgt[:])     # S * tgt cosine
            nc.scalar.activation(e2[:], t1[:], AF.Exp)
            # exp(t1 - SM) == exp(t1) * exp(-SM); const bias APs for -SM
            # aren't registered, so fold via DVE scalar-mul instead
            nc.vector.tensor_scalar_mul(e1[:], e2[:], float(np.exp(-SM)))
            # (tot - pad_corr) - e2
            nc.vector.scalar_tensor_tensor(out=den[:], in0=tot[:],
                                           scalar=-pad_corr, in1=e2[:],
                                           op0=ALU.add, op1=ALU.subtract)
            nc.vector.tensor_add(den[:], den[:], e1[:])
            nc.scalar.activation(lg[:], den[:], AF.Ln)
            nc.vector.tensor_sub(v[:], lg[:], t1[:])
            nc.vector.reduce_sum(rowv[:], v[:], axis=AX.X)
            pss = psp1.tile([1, 1], f32, tag="pss", name="pss")
            nc.tensor.matmul(pss[:], rowv[:], ones[:], start=True, stop=True)
            nc.vector.tensor_scalar_mul(res[:], pss[:], 1.0 / n)
            nc.vector.tensor_scalar_add(res[:], res[:], SM)
            nc.sync.dma_start(out[:], res[:])

    nc.compile()
    return nc


def in_maps(x, W, labels, c_shard=C_SHARD, ct=CT, nj=NJ, n_cores=N_CORES):
    d = x.shape[1]
    c_pad = nj * ct
    x = np.ascontiguousarray(np.asarray(x, dtype=np.float32))
    W = np.ascontiguousarray(np.asarray(W, dtype=np.float32))
    lab = np.asarray(labels).astype(np.int64)
    xt = np.ascontiguousarray(x.T)
    wlg = np.ascontiguousarray(W[lab])
    maps = []
    for c in range(n_cores):
        wt = np.zeros((d, c_pad), np.float32)
        wt[:, :c_shard] = W[c * c_shard:(c + 1) * c_shard].T
        maps.append({"x_nat": x, "xt": xt, "wl": wlg, "wt": wt})
    return maps


_CACHE = {}


def _get_nc():
    if "nc" not in _CACHE:
        _CACHE["nc"] = build()
    return _CACHE["nc"]


def kernel(x, W, labels):
    nc = _get_nc()
    res = run_bass_kernel_spmd(nc, in_maps(x, W, labels),
                               core_ids=list(range(N_CORES)))
    val = np.asarray(res.results[0]["out"], dtype=np.float32)
    return val.reshape(())
